# revision 26
# baseline (speedup 1.0000x reference)
"""Trainium2 Bass kernel for a GAT-style attention head.

Reference computation (B=1, C=512, N=8192, F=256):
    seq_fts = einsum('bcn,fc->bfn', x, W1)                  # [1,F,N]
    f1 = seq_fts . w21 + b21 ;  f2 = seq_fts . w22 + b22    # [1,N]
    logits[i,j] = f1[j] + f2[i]  masked by adj>0 (else -1e9)
    logits = leaky_relu(logits, 0.01)
    coefs = softmax(logits, axis=1)        # normalises over i for each j
    ret[i,f] = sum_j coefs[i,j]*seq_fts[f,j] + bias[f]
    out = elu(ret).transpose -> [1,F,N]

Distribution: shard rows i across 8 NeuronCores (1024 rows each).  The
softmax denominator D[j] = sum_i E[i,j] (E = exp of masked leaky-relu
logits) is indexed by the *contracted* axis j, so each core computes a
partial D over its rows, chunked 32KB AllReduces produce the full D
(pipelined with compute), and 1/D is folded into the seq_fts columns
before the local matmul
    out[f, i_blk] = sum_j (seqT[j,f]/D[j]) * E[j, i_blk].

Per-core pipeline (E kept transposed, [j on partitions, i free]):
  - a tiny warm-up AllGather is fired at t=0 to absorb the one-time
    collective-stack spin-up (~50us) off the critical path.
  - f1/f2 via plain-fp16 matmuls against x_own (error ~1e-3 on logits,
    well inside tolerance); f1 is AllGathered (32KB) and transposed
    into per-partition bias layout via the PE transpose path.
  - seqT (fp16) computed on the own n-block and AllGathered (4MB),
    overlapping the elementwise phase.
  - elementwise phase instruction-merged: the fp16 {0,-8192} additive
    mask + f2 broadcast are combined in quad-tile TENSOR_TENSORs
    ([128, 4x1024], 2x DVE mode, one DMA per quad); per tile either
    ScalarE Prelu(bias=f1[j], alpha=.01) or a DVE pair (fused
    two-scalar tensor_scalar + TT max) for load balancing; ScalarE
    Exp (bias -ln16) writes et in fp8-e4m3 with accum_out emitting
    partial D/16 per tile.
  - 5-way chunked AllReduce of D/16; per chunk 4096/D is broadcast
    via a stride-0 DMA and folded into the fp8 seqd slab with ONE
    chunk-wide TT; fp8 x fp8 matmuls accumulate 256*out[f,i] in PSUM.
  - epilogue: scale 1/256 + bias + ELU via relu(x)+exp(min(x,0))-1.
"""

import os
import sys

if "/opt/trn_rl_repo" not in sys.path:
    sys.path.insert(0, "/opt/trn_rl_repo")

import numpy as np

import concourse.bass as bass
import concourse.tile as tile
from concourse import bacc, mybir

F32 = mybir.dt.float32
F16 = mybir.dt.float16
F8 = mybir.dt.float8e4

B, C, N, F = 1, 512, 8192, 256
NCORES = 8
NB = N // NCORES          # rows per core (i block)
P = 128
NJT = N // P              # 64 j tiles
NS = NB // P              # 8 n sub-tiles per core
CO = C // P               # 4 contraction tiles for seq_fts
NCHUNK = 6                # allreduce chunks
CH = [4, 12, 12, 12, 12, 12]      # j-tiles per chunk (small head chunk)
CH0 = [0, 4, 16, 28, 40, 52]      # chunk start offsets
ETR = 12                  # rotating et tile slot count (>= max CH[1:])
NHEAD = 16                # j-tiles whose f1 every core computes locally
# ag2 seqt blocks (of 8 j-tiles) first needed per chunk
SEQT_BLOCKS = [[0], [1], [2, 3], [4], [5, 6], [7]]
MM_N = 512                # moving free dim per matmul

AF = mybir.ActivationFunctionType
OP = mybir.AluOpType

_PROGRAM_CACHE = {}
LAST_RESULTS = None       # BassKernelResults of the most recent run (for test.py)


def _is_act_quad(q: int) -> bool:
    """ScalarE-Prelu path for 11/16 quads (44/64 tiles); DVE leaky for 5."""
    return q % 16 not in (2, 5, 8, 11, 14)


def _build_program(b21f: float, b22f: float):
    nc = bacc.Bacc("TRN2", target_bir_lowering=False, debug=False,
                   num_devices=NCORES)

    # ---- per-core external inputs -------------------------------------
    xb_t = nc.dram_tensor("xb", [C, NB], F32, kind="ExternalInput")
    xh_t = nc.dram_tensor("xh", [C, NHEAD * P], F32, kind="ExternalInput")
    w1t_t = nc.dram_tensor("w1t", [C, F], F32, kind="ExternalInput")
    w21_t = nc.dram_tensor("w21", [1, F], F32, kind="ExternalInput")
    w22_t = nc.dram_tensor("w22", [1, F], F32, kind="ExternalInput")
    bias_t = nc.dram_tensor("bias", [F], F32, kind="ExternalInput")
    id_t = nc.dram_tensor("ident", [P, P], F32, kind="ExternalInput")
    mk_t = nc.dram_tensor("mk", [N, NB], F16, kind="ExternalInput")
    out_t = nc.dram_tensor("outb", [F, NB], F32, kind="ExternalOutput")

    groups = [list(range(NCORES))]

    with tile.TileContext(nc) as tc:
        with tc.tile_pool(name="dram", bufs=1, space="DRAM") as dram:
            wu_in = dram.tile([8], F32, name="wu_in")
            wu_out = dram.tile([64], F32, name="wu_out", addr_space="Shared")
            ag1_in = dram.tile([NB], F32, name="ag1_in")
            ag1_out = dram.tile([N], F32, name="ag1_out", addr_space="Shared")
            ag2_in = dram.tile([NB * F], F16, name="ag2_in")
            ag2_out = dram.tile([N * F], F16, name="ag2_out",
                                addr_space="Shared")
            f2tmp = dram.tile([NB], F16, name="f2tmp")
            f1h_d = dram.tile([NHEAD * P], F32, name="f1h_d")
            GW = [CH[0] + CH[1], CH[2] + CH[3], CH[4], CH[5]]
            ar_in = [dram.tile([P * GW[g]], F32, name=f"ar_in{g}")
                     for g in range(4)]
            ar_out = [dram.tile([P * GW[g]], F32, name=f"ar_out{g}",
                                addr_space="Shared") for g in range(4)]

            # ---------- persistent SBUF ----------
            with tc.tile_pool(name="persist", bufs=1) as persist:
                seqt = persist.tile([P, NJT, F], F16, name="seqt")
                f2b16q = persist.tile([P, 2, NB], F16, name="f2b16q")
                f1colh = persist.tile([P, NHEAD], F32, name="f1colh")
                f1colt = persist.tile([P, NJT - NHEAD], F32, name="f1colt")
                bias_sb = persist.tile([P, F // P], F32, name="bias_sb")
                ident = persist.tile([P, P], F32, name="ident")

                # ---------- phase 0 ----------
                with tc.tile_pool(name="p0", bufs=1) as p0, \
                     tc.tile_pool(name="p0ps", bufs=2, space="PSUM") as p0ps:
                    # warm-up collective: absorbs the one-time collective
                    # stack spin-up while phase 0 runs.
                    x_sb = p0.tile([P, CO, NB], F32, name="x_sb")
                    nc.sync.dma_start(
                        x_sb[:],
                        xb_t.ap().rearrange("(co ci) n -> ci co n", ci=P))
                    w1t_sb = p0.tile([P, CO, F], F32, name="w1t_sb")
                    nc.sync.dma_start(
                        w1t_sb[:],
                        w1t_t.ap().rearrange("(co ci) f -> ci co f", ci=P))
                    w21b = p0.tile([P, F], F32, name="w21b")
                    nc.sync.dma_start(w21b[:],
                                      w21_t.ap()[0:1, :].to_broadcast((P, F)))
                    w22b = p0.tile([P, F], F32, name="w22b")
                    nc.sync.dma_start(w22b[:],
                                      w22_t.ap()[0:1, :].to_broadcast((P, F)))
                    nc.sync.dma_start(
                        bias_sb[:],
                        bias_t.ap().rearrange("(ft fi) -> fi ft", fi=P))
                    nc.sync.dma_start(ident[:], id_t.ap())

                    # fp16 copies of x and W1 (feed f1/f2 and seqT matmuls)
                    xho = p0.tile([P, CO, NB], F16, name="xho")
                    nc.vector.tensor_copy(xho[:], x_sb[:])
                    w1h = p0.tile([P, CO, F], F16, name="w1h")
                    nc.vector.tensor_copy(w1h[:], w1t_sb[:])

                    # u1/u2 = W1^T w21 / w22  (fp32, c on partitions)
                    u_sb = p0.tile([P, CO, 2], F32, name="u_sb")
                    for co in range(CO):
                        tu = p0.tile([P, F], F32, name="tu", tag="tu")
                        nc.vector.tensor_tensor(tu[:], w1t_sb[:, co, :],
                                                w21b[:], OP.mult)
                        nc.vector.tensor_reduce(u_sb[:, co, 0:1], tu[:],
                                                mybir.AxisListType.X, OP.add)
                        tv = p0.tile([P, F], F32, name="tv", tag="tv")
                        nc.vector.tensor_tensor(tv[:], w1t_sb[:, co, :],
                                                w22b[:], OP.mult)
                        nc.vector.tensor_reduce(u_sb[:, co, 1:2], tv[:],
                                                mybir.AxisListType.X, OP.add)
                    u16 = p0.tile([P, CO, 2], F16, name="u16")
                    nc.vector.tensor_copy(u16[:], u_sb[:])

                    # f1/f2 on own block (plain fp16: |err| ~1e-3 on logits)
                    f1ps = p0ps.tile([1, NB], F32, name="f1ps",
                                     tag="psrow", bufs=2)
                    f2ps = p0ps.tile([1, NB], F32, name="f2ps",
                                     tag="psrow", bufs=2)
                    for q, ps in ((1, f2ps), (0, f1ps)):
                        for ih in range(2):
                            sl = slice(ih * MM_N, (ih + 1) * MM_N)
                            for co in range(CO):
                                nc.tensor.matmul(
                                    ps[:, sl], lhsT=u16[:, co, q:q + 1],
                                    rhs=xho[:, co, sl],
                                    start=(co == 0), stop=(co == CO - 1))
                    f1row = p0.tile([1, NB], F32, name="f1row")
                    nc.vector.tensor_scalar_add(f1row[:], f1ps[:], b21f)
                    nc.sync.dma_start(ag1_in[:].rearrange("n -> () n"),
                                      f1row[:])
                    # small AllGather: f1 (32KB total) — fire early
                    nc.gpsimd.collective_compute(
                        "AllGather", OP.bypass, replica_groups=groups,
                        ins=[ag1_in.opt()], outs=[ag1_out.opt()])

                    # local f1 for the first NHEAD j-tiles (every core holds
                    # x[:, :NHEAD*128]): unblocks elementwise ~40us before
                    # the AllGathered f1 lands.
                    xh_sb = p0.tile([P, CO, NHEAD * P], F32, name="xh_sb")
                    nc.sync.dma_start(
                        xh_sb[:],
                        xh_t.ap().rearrange("(co ci) n -> ci co n", ci=P))
                    xh16 = p0.tile([P, CO, NHEAD * P], F16, name="xh16")
                    nc.vector.tensor_copy(xh16[:], xh_sb[:])
                    for half in range(2):
                        h0 = half * (NHEAD * P // 2)
                        f1hps = p0ps.tile([1, NHEAD * P // 2], F32,
                                          name="f1hps", tag="psrow", bufs=2)
                        for ih in range(2):
                            sl = slice(h0 + ih * MM_N, h0 + (ih + 1) * MM_N)
                            psl = slice(ih * MM_N, (ih + 1) * MM_N)
                            for co in range(CO):
                                nc.tensor.matmul(
                                    f1hps[:, psl], lhsT=u16[:, co, 0:1],
                                    rhs=xh16[:, co, sl],
                                    start=(co == 0), stop=(co == CO - 1))
                        f1hrow = p0.tile([1, NHEAD * P // 2], F32,
                                         name="f1hrow", tag="f1hrow")
                        nc.vector.tensor_scalar_add(f1hrow[:], f1hps[:], b21f)
                        nc.sync.dma_start(
                            f1h_d[h0:h0 + NHEAD * P // 2]
                            .rearrange("n -> () n"), f1hrow[:])
                    # f1colh columns: tiny partition-major DMAs (no PE
                    # transpose, no PSUM->SBUF copy, no engine queues)
                    for t in range(NHEAD):
                        nc.sync.dma_start(
                            f1colh[:, t:t + 1],
                            f1h_d[t * P:(t + 1) * P].rearrange("n -> n ()"))

                    f2row = p0.tile([1, NB], F16, name="f2row")
                    nc.vector.tensor_scalar_add(f2row[:], f2ps[:], b22f)
                    nc.sync.dma_start(f2tmp[:].rearrange("n -> () n"),
                                      f2row[:])
                    nc.sync.dma_start(
                        f2b16q[:],
                        f2tmp[None, None, :].to_broadcast((P, 2, NB)))


                    # seqT (own block)
                    seqtown = p0.tile([P, NS, F], F16, name="seqtown")
                    for ns in range(NS):
                        sps = p0ps.tile([P, F], F32, name="sps", tag="sps")
                        for co in range(CO):
                            nc.tensor.matmul(
                                sps[:],
                                lhsT=xho[:, co, ns * P:(ns + 1) * P],
                                rhs=w1h[:, co, :],
                                start=(co == 0), stop=(co == CO - 1))
                        nc.vector.tensor_copy(seqtown[:, ns, :], sps[:])
                    nc.sync.dma_start(
                        ag2_in.rearrange("(ci ns f) -> ci ns f",
                                         ci=P, ns=NS),
                        seqtown[:])

                # ---------- main loop ----------
                with tc.tile_pool(name="etpool", bufs=1) as etp, \
                     tc.tile_pool(name="stream", bufs=3) as stream, \
                     tc.tile_pool(name="dtiles", bufs=1) as dtiles, \
                     tc.tile_pool(name="outps", bufs=1, space="PSUM") as outps, \
                     tc.tile_pool(name="epil", bufs=1) as epil:

                    out_ps = [outps.tile([P, MM_N], F32, name=f"out_ps{q}",
                                         tag=f"out_ps{q}")
                              for q in range(4)]
                    dp_c = [dtiles.tile([P, CH[k]], F32, name=f"dp{k}",
                                        tag=f"dp{k}") for k in range(NCHUNK)]
                    inv_of = {}

                    et_by_k = {}
                    mm_pending = []

                    def _emit_mms(kk, et_t):
                        for jl in range(CH[kk]):
                            jt = CH0[kk] + jl
                            sd = stream.tile([P, F], F16, name="seqd",
                                             tag="seqd", bufs=4)
                            inv_g, off = inv_of[kk]
                            nc.vector.tensor_scalar_mul(
                                sd[:], seqt[:, jt, :],
                                inv_g[:, off + jl:off + jl + 1])
                            for fi in range(2):
                                for ih in range(2):
                                    nc.tensor.matmul(
                                        out_ps[fi * 2 + ih][:],
                                        lhsT=sd[:, fi * P:(fi + 1) * P],
                                        rhs=et_t[:, jl,
                                               ih * MM_N:(ih + 1) * MM_N],
                                        start=(jt == 0), stop=(jt == NJT - 1))

                    for k in range(NCHUNK):
                        if k == 2:
                            # seqT AllGather: issued after AR0/AR1 so the
                            # cc order is ag1, AR0, AR1, ag2, AR2..; the
                            # seqt loads ride the gpsimd queue right after
                            # ag2's completion wait (data-ready then).
                            nc.gpsimd.collective_compute(
                                "AllGather", OP.bypass,
                                replica_groups=groups,
                                ins=[ag2_in.opt()], outs=[ag2_out.opt()])
                            srcv = ag2_out.rearrange(
                                "(b ci ns f) -> b ci ns f",
                                b=NCORES, ci=P, ns=NS)
                            for b in range(NCORES):
                                nc.gpsimd.dma_start(
                                    seqt[:, b * NS:(b + 1) * NS, :], srcv[b])
                            # f1col tail from the AllGathered f1 —
                            # TensorE-queue DMA + transpose, VectorE copy:
                            # none of these block the head tiles' Prelu/Exp
                            # stream on ScalarE.
                            t64 = dtiles.tile([NJT - NHEAD, P], F32,
                                              name="t64")
                            nc.sync.dma_start(
                                t64[:],
                                ag1_out.rearrange("(jt jp) -> jt jp", jp=P)
                                [NHEAD:NJT, :])
                            tps = outps.tile([P, NJT - NHEAD], F32,
                                             name="tps")
                            nc.tensor.matmul(
                                tps[:], lhsT=t64[:],
                                rhs=ident[:NJT - NHEAD, :NJT - NHEAD],
                                is_transpose=True, start=True, stop=True)
                            nc.vector.tensor_copy(f1colt[:], tps[:])
                        # et tile: chunks 0/1 have dedicated slots (no
                        # rotation stall at pipeline head); chunks 2+ rotate
                        # 3 buffers, freed after each chunk's matmuls.
                        if k == 0:
                            et_k = etp.tile([P, CH[0], NB], F16, name="et0",
                                            tag="et0")
                        else:
                            et_k = etp.tile([P, ETR, NB], F16, name="etr",
                                            tag="etr", bufs=4)
                        for jl in range(0, CH[k], 2):
                            jt = CH0[k] + jl
                            # one DMA + one [128,2048] 2x-mode TT applies
                            # mask + f2 for 2 tiles at once; even pairs are
                            # ScalarE-Prelu tiles, odd pairs DVE-leaky tiles
                            # (pair-wide max TT)
                            mkq = stream.tile([P, 2, NB], F16, name="mkq",
                                              tag="mkq", bufs=4)
                            nc.sync.dma_start(
                                mkq[:],
                                mk_t.ap()[jt * P:(jt + 2) * P, :]
                                .rearrange("(jl jp) i -> jp jl i", jp=P))
                            lt2q = stream.tile([P, 2, NB], F16,
                                               name="lt2q", tag="lt2q",
                                               bufs=3)
                            nc.vector.tensor_tensor(lt2q[:], f2b16q[:],
                                                    mkq[:], OP.add)
                            ltq = stream.tile([P, 2, NB], F16, name="ltq",
                                              tag="ltq", bufs=2)
                            f1c = [(f1colh[:, jt + t:jt + t + 1]
                                    if jt + t < NHEAD
                                    else f1colt[:, jt + t - NHEAD:
                                                jt + t - NHEAD + 1])
                                   for t in range(2)]
                            if (jt // 2) % 8 in (0, 3, 6):
                                for jq in range(2):
                                    nc.scalar.activation(
                                        ltq[:, jq, :], lt2q[:, jq, :],
                                        AF.Prelu, bias=f1c[jq],
                                        scale=1.0, alpha=0.01)
                            else:
                                stq = stream.tile([P, 2, NB], F16,
                                                  name="stq",
                                                  tag="stq", bufs=1)
                                s2q = stream.tile([P, 2, NB], F16,
                                                  name="s2q",
                                                  tag="s2q", bufs=1)
                                for jq in range(2):
                                    nc.vector.tensor_scalar_add(
                                        stq[:, jq, :], lt2q[:, jq, :],
                                        f1c[jq])
                                    nc.vector.tensor_scalar(
                                        s2q[:, jq, :], lt2q[:, jq, :],
                                        f1c[jq], 0.01, OP.add, OP.mult)
                                nc.vector.tensor_tensor(
                                    ltq[:], stq[:], s2q[:], OP.max)
                            # per-tile Exp writes et; accum_out -> partial D
                            for jq in range(2):
                                nc.scalar.activation(
                                    et_k[:, jl + jq, :], ltq[:, jq, :],
                                    AF.Exp,
                                    accum_out=dp_c[k][:, jl + jq:jl + jq + 1])

                        # grouped AllReduce of partial D: chunk pairs for
                        # 0+1 / 2+3, per-chunk for 4 and 5 (small late ARs
                        # shrink the end-of-kernel matmul burst)
                        if k in (1, 3, 4, 5):
                            g = {1: 0, 3: 1, 4: 2, 5: 3}[k]
                            paired = k in (1, 3)
                            arv = ar_in[g].rearrange("(jp jl) -> jp jl",
                                                     jp=P)
                            if paired:
                                nc.gpsimd.dma_start(arv[:, 0:CH[k - 1]],
                                                    dp_c[k - 1][:])
                                nc.gpsimd.dma_start(arv[:, CH[k - 1]:GW[g]],
                                                    dp_c[k][:])
                            else:
                                nc.gpsimd.dma_start(arv[:], dp_c[k][:])
                            nc.gpsimd.collective_compute(
                                "AllReduce", OP.add, replica_groups=groups,
                                ins=[ar_in[g].opt()],
                                outs=[ar_out[g].opt()])
                            dsum = dtiles.tile([P, GW[g]], F32,
                                               name=f"dsum{g}",
                                               tag=f"dsum{g}")
                            nc.gpsimd.dma_start(
                                dsum[:],
                                ar_out[g].rearrange("(jp jl) -> jp jl",
                                                    jp=P))
                            inv_g = dtiles.tile([P, GW[g]], F32,
                                                name=f"invg{g}",
                                                tag=f"invg{g}")
                            nc.vector.reciprocal(inv_g[:], dsum[:])
                            if paired:
                                inv_of[k - 1] = (inv_g, 0)
                                inv_of[k] = (inv_g, CH[k - 1])
                            else:
                                inv_of[k] = (inv_g, 0)

                        # MM emission: chunks 0/1 after the k==2 seqt
                        # loads; chunks 2..5 after their pair's AllReduce.
                        et_by_k[k] = et_k
                        mm_pending.append(k)
                        if k >= 2:
                            ready = [kk for kk in mm_pending
                                     if kk in inv_of]
                            for kk in ready:
                                _emit_mms(kk, et_by_k[kk])
                                mm_pending.remove(kk)

                    # ---------- epilogue: 1/256 scale + bias + ELU ----------
                    # elu(t) = relu(t) + exp(min(t,0)) - 1
                    for fi in range(2):
                        for ih in range(2):
                            ps = out_ps[fi * 2 + ih]
                            t = epil.tile([P, MM_N], F32, name="t", tag="ep_t",
                                          bufs=1)
                            nc.scalar.activation(t[:], ps[:], AF.Identity,
                                                 bias=bias_sb[:, fi:fi + 1],
                                                 scale=1.0)
                            m = epil.tile([P, MM_N], F32, name="m", tag="ep_m",
                                          bufs=1)
                            nc.vector.tensor_scalar_min(m[:], t[:], 0.0)
                            nc.vector.tensor_scalar_max(t[:], t[:], 0.0)
                            nc.scalar.activation(m[:], m[:], AF.Exp)
                            nc.vector.scalar_tensor_tensor(
                                m[:], m[:], -1.0, t[:], OP.add, OP.add)
                            nc.sync.dma_start(
                                out_t.ap()[fi * P:(fi + 1) * P,
                                           ih * MM_N:(ih + 1) * MM_N], m[:])

    nc.compile()
    return nc


def kernel(x, adj, W1, w21, b21, w22, b22, bias):
    global LAST_RESULTS
    from concourse.bass_utils import run_bass_kernel_spmd

    x = np.asarray(x)
    adj = np.asarray(adj)
    W1 = np.asarray(W1, dtype=np.float32)
    w21 = np.asarray(w21, dtype=np.float32)
    w22 = np.asarray(w22, dtype=np.float32)
    bias = np.asarray(bias, dtype=np.float32)
    b21f = float(np.asarray(b21))
    b22f = float(np.asarray(b22))

    key = (b21f, b22f)
    if key not in _PROGRAM_CACHE:
        _PROGRAM_CACHE[key] = _build_program(b21f, b22f)
    nc = _PROGRAM_CACHE[key]

    w1t = np.ascontiguousarray(W1.T)                      # [C, F]
    xh = np.ascontiguousarray(x[0, :, 0:NHEAD * P], dtype=np.float32)
    identity = np.eye(P, dtype=np.float32)
    in_maps = []
    for c in range(NCORES):
        blk = slice(c * NB, (c + 1) * NB)
        xb = np.ascontiguousarray(x[0, :, blk], dtype=np.float32)
        # additive mask, transposed: [j, i_local] fp16 {0, -8192}
        mk = ((adj[0, blk, :].T.astype(np.float32) - 1.0)
              * 8192.0).astype(np.float16)
        in_maps.append({
            "xb": xb,
            "xh": xh,
            "ident": identity,
            "w1t": w1t,
            "w21": w21.reshape(1, F),
            "w22": w22.reshape(1, F),
            "bias": bias,
            "mk": mk,
        })

    trace = os.environ.get("BASS_KERNEL_TRACE") == "1"
    kwargs = {}
    if trace:
        _install_ntff_hook()
        import concourse.bass_utils as bu
        bu.upload_artifacts = lambda d: d          # no S3 in this sandbox
        kwargs = dict(trace=True, trace_cores=list(range(NCORES)),
                      tmpdir=os.environ.get("BASS_KERNEL_TRACE_DIR"))

    res = run_bass_kernel_spmd(nc, in_maps, core_ids=list(range(NCORES)),
                               **kwargs)
    LAST_RESULTS = res

    out = np.empty((B, F, N), dtype=np.float32)
    for c in range(NCORES):
        out[0, :, c * NB:(c + 1) * NB] = res.results[c]["outb"]
    return out


def _install_ntff_hook():
    """Register the axon NTFF profiling hook (missing antenv.axon_hooks)."""
    import types
    import contextlib
    import ctypes

    if "antenv.axon_hooks" in sys.modules:
        return
    so_path = "/opt/axon/libaxon_pjrt.so"
    lib = ctypes.CDLL(so_path)
    if not hasattr(lib, "axon_start_nrt_profile"):
        return
    lib.axon_start_nrt_profile.argtypes = [ctypes.POINTER(ctypes.c_int64),
                                           ctypes.c_size_t]
    lib.axon_start_nrt_profile.restype = ctypes.c_int64
    lib.axon_stop_nrt_profile.argtypes = [ctypes.c_char_p]
    lib.axon_stop_nrt_profile.restype = ctypes.c_int64

    @contextlib.contextmanager
    def _hook(output_dir, device_ids):
        import jax
        jax.devices()
        if device_ids:
            ids = (ctypes.c_int64 * len(device_ids))(*device_ids)
            rc = lib.axon_start_nrt_profile(ids, len(device_ids))
        else:
            rc = lib.axon_start_nrt_profile(None, 0)
        if rc != 0:
            raise RuntimeError(f"axon_start_nrt_profile rc={rc}")
        try:
            yield
        finally:
            n = lib.axon_stop_nrt_profile(str(output_dir).encode())
            print(f"ntff profile: {n} file(s) -> {output_dir}",
                  file=sys.stderr)

    mod = types.ModuleType("antenv.axon_hooks")
    mod.get_axon_ntff_profile_hook = lambda: _hook
    mod.set_axon_ntff_profile_hook = lambda h: None
    sys.modules["antenv.axon_hooks"] = mod
